# revision 16
# baseline (speedup 1.0000x reference)
"""Distributed Trainium2 kernel for the gated-adapter attention module.

Head-parallel tensor parallelism over 8 NeuronCores (4 heads each).
Host-side prep (inside kernel()): inputs are pre-transposed and
pre-cast to bf16 so the device never transposes weights or x —
xT [D, TOK], wqT/wkT/wvT [D, CH], and the full wo^T [D, D] arrive
matmul-ready.  The adapter K/V projections and tanh(gate) (folded
into av) are precomputed on host.

Device pipeline per core:
  A) QKV over 256-token panels: q/k emitted w-stationary straight
     into [ch, tok] orientation (no PE transposes); RoPE applied in
     that orientation with a pair-swap permutation matmul plus wide
     DVE multiply/adds against host-precomputed cos / signed-sin
     tables; v emitted x-stationary natural [tok, ch].  q^T/k^T/v
     spill to DRAM for phase B.
  B) Attention per (batch, head) in S^T orientation: scores [k, q],
     exp on ACT batched two 128-k tiles per instruction, 0/1 diagonal
     masks on DVE, softmax sums via a running DVE pair-add
     accumulator + one ones-matmul per 512-query chunk, gated adapter
     branch (gate pre-folded into av).  Each chunk's PV matmuls,
     adapter matmuls and normalization tail are emitted one chunk
     behind the chunk's score matmuls (proportional interleave), so
     PE instructions always have completed producers.
  C) RowParallel wo: yT = o^T-chunks @ wo quarters; wo quarters 0/1
     are preloaded during phase B into a pool that survives into
     phase C, and all batch-0 token chains run first so the b1
     AllToAll is fully hidden.

Engine map: sync = x prefetch + phase-C wqt2/3; scalar = wq + smalls
+ spills/exp; vector = wk loads + DVE compute; gpsimd = wv loads,
phase-B loads, wqt0/1 preloads, collectives.
"""

import sys

sys.path.insert(0, "/opt/trn_rl_repo")

import numpy as np
import ml_dtypes

import concourse.bass as bass
import concourse.mybir as mybir
import concourse.tile as tile
from concourse import bacc, bass_utils
from concourse.bass import ds, ts

N_CORES = 8
B, S, D = 2, 2048, 4096
H = 32
HD = 128                      # head dim
H_LOC = H // N_CORES          # 4 heads per core
CH = H_LOC * HD               # 512 local channels
TOK = B * S                   # 4096 tokens
NK = D // 128                 # 32 contraction tiles
AL = 10                       # adapter length
NQC = S // 512                # 4 query chunks per sequence
NPAN = TOK // 256             # 16 phase-A token panels
NDQ = 4                       # dout chunks for the wo pipeline
DQW = D // NDQ                # 1024 dout rows per chunk
SCALE = 1.0 / float(np.sqrt(HD))
BF = mybir.dt.bfloat16
F32 = mybir.dt.float32
EXP = mybir.ActivationFunctionType.Exp
COPY = mybir.ActivationFunctionType.Copy
BF_NP = ml_dtypes.bfloat16


def build():
    nc = bacc.Bacc("TRN2", target_bir_lowering=False, debug=False,
                   num_devices=N_CORES)
    xT = nc.dram_tensor("xT", [D, TOK], BF, kind="ExternalInput")
    wqT = nc.dram_tensor("wqT", [D, CH], BF, kind="ExternalInput")
    wkT = nc.dram_tensor("wkT", [D, CH], BF, kind="ExternalInput")
    wvT = nc.dram_tensor("wvT", [D, CH], BF, kind="ExternalInput")
    woTF = nc.dram_tensor("woTF", [D, D], BF, kind="ExternalInput")
    csT = nc.dram_tensor("csT", [128, S], BF, kind="ExternalInput")
    snP = nc.dram_tensor("snP", [128, S], BF, kind="ExternalInput")
    psw = nc.dram_tensor("psw", [128, 128], BF, kind="ExternalInput")
    akT = nc.dram_tensor("akT", [HD, H_LOC * AL], BF, kind="ExternalInput")
    av = nc.dram_tensor("av", [AL, CH], BF, kind="ExternalInput")
    m01 = nc.dram_tensor("m01", [128, 4 * 512], BF, kind="ExternalInput")
    out = nc.dram_tensor("out", [TOK // N_CORES, D], F32,
                         kind="ExternalOutput")

    with tile.TileContext(nc) as tc:
        with tc.tile_pool(name="dram", bufs=1, space="DRAM") as dram, \
             tc.tile_pool(name="persist", bufs=1) as persist:
            qT_d = dram.tile([CH, TOK], BF, tag="qT_d")
            kT_d = dram.tile([CH, TOK], BF, tag="kT_d")
            v_d = dram.tile([TOK, CH], BF, tag="v_d")
            a2a_ins = [dram.tile([N_CORES, CH, 256], BF, tag=f"a2ai{b}",
                                 name=f"a2ai{b}") for b in range(B)]
            a2a_outs = [dram.tile([N_CORES, CH, 256], BF, tag=f"a2ao{b}",
                                  name=f"a2ao{b}") for b in range(B)]

            oTf = persist.tile([128, NK, TOK // N_CORES], BF, tag="oTf")
            ones = persist.tile([128, 128], BF, tag="ones")
            nc.vector.memset(ones[:], 1.0)
            akT_sb = persist.tile([128, H_LOC, AL], BF, tag="akT_sb")
            av_sb = persist.tile([AL, CH], BF, tag="av_sb")
            m01_sb = persist.tile([128, 4, 512], BF, tag="m01_sb")

            # ================= phase A: QKV =================
            with tc.tile_pool(name="wres", bufs=1) as wres, \
                 tc.tile_pool(name="xa", bufs=2) as xa, \
                 tc.tile_pool(name="ar", bufs=4) as ar, \
                 tc.tile_pool(name="ps_qk", bufs=3, space="PSUM") as ps_qk, \
                 tc.tile_pool(name="ps_sw", bufs=2, space="PSUM") as ps_sw, \
                 tc.tile_pool(name="ps_v", bufs=2, space="PSUM") as ps_v:
                wTs = [wres.tile([128, NK, CH], BF, tag=f"wT{nm}",
                                 name=f"wT{nm}") for nm in ("q", "k", "v")]
                # phase-A-only tables live in wres so they free with it
                psw_sb = wres.tile([128, 128], BF, tag="psw_sb")
                cs_sb = wres.tile([128, S], BF, tag="cs_sb")
                sn_sb = wres.tile([128, S], BF, tag="sn_sb")
                # weights split across the scalar and gpsimd queues,
                # quarter-chunked; wq and wk stream first (q/k chains
                # lead each panel), wv follows on gpsimd
                for cq in range(4):
                    for wT, wt, eng in ((wTs[0], wqT, nc.scalar),
                                        (wTs[1], wkT, nc.gpsimd)):
                        eng.dma_start(
                            wT[:, ds(cq * (NK // 4), NK // 4), :],
                            wt.ap()[ds(cq * (D // 4), D // 4), :].rearrange(
                                "(nk p) c -> p nk c", p=128))
                    if cq == 0:
                        # smalls ride scalar after the first wq quarter
                        nc.scalar.dma_start(psw_sb[:], psw.ap())
                        nc.scalar.dma_start(cs_sb[:], csT.ap())
                        nc.scalar.dma_start(sn_sb[:], snP.ap())
                        nc.scalar.dma_start(
                            m01_sb[:],
                            m01.ap().rearrange("p (j q) -> p j q", j=4))
                        nc.scalar.dma_start(
                            akT_sb[:],
                            akT.ap().rearrange("p (h a) -> p h a", h=H_LOC))
                        nc.scalar.dma_start(av_sb[:], av.ap())
                for cq in range(4):
                    nc.gpsimd.dma_start(
                        wTs[2][:, ds(cq * (NK // 4), NK // 4), :],
                        wvT.ap()[ds(cq * (D // 4), D // 4), :].rearrange(
                            "(nk p) c -> p nk c", p=128))

                for pan in range(NPAN):
                    xt = xa.tile([128, NK, 256], BF, tag="xt")
                    for qd in range(4):
                        nc.sync.dma_start(
                            xt[:, ds(qd * (NK // 4), NK // 4), :],
                            xT.ap()[ds(qd * (D // 4), D // 4),
                                    ds(pan * 256, 256)].rearrange(
                                "(k p) t -> p k t", p=128))
                    sb0 = (pan % (NPAN // B)) * 256   # seq offset in batch
                    csl = cs_sb[:, ds(sb0, 256)]
                    ssl = sn_sb[:, ds(sb0, 256)]
                    swap_pend = []

                    def qk_chain(wi, ct, dst, xt=xt, csl=csl, ssl=ssl,
                                 pan=pan, swap_pend=swap_pend):
                        pp = ps_qk.tile([128, 256], F32, tag="ppqk")
                        for dt in range(NK):
                            nc.tensor.matmul(pp[:],
                                             lhsT=wTs[wi][:, dt, ts(ct, 128)],
                                             rhs=xt[:, dt, :],
                                             start=(dt == 0),
                                             stop=(dt == NK - 1))
                        u = ar.tile([128, 256], BF, tag="u")
                        nc.vector.tensor_mul(u[:], pp[:], ssl)
                        t2 = ar.tile([128, 256], F32, tag="t2")
                        nc.vector.tensor_mul(t2[:], pp[:], csl)

                        def _swap(u=u, t2=t2, ct=ct, dst=dst, pan=pan):
                            sw = ps_sw.tile([128, 256], F32, tag="sw")
                            nc.tensor.matmul(sw[:], lhsT=psw_sb[:], rhs=u[:],
                                             start=True, stop=True)
                            qb_ = ar.tile([128, 256], BF, tag="qb")
                            nc.vector.tensor_add(qb_[:], t2[:], sw[:])
                            nc.scalar.dma_start(
                                dst[ds(ct * 128, 128), ds(pan * 256, 256)],
                                qb_[:])
                        swap_pend.append(_swap)

                    def v_chain(tk, xt=xt, pan=pan):
                        pv_ = ps_v.tile([128, CH], F32, tag="ppv")
                        for dt in range(NK):
                            nc.tensor.matmul(pv_[:],
                                             lhsT=xt[:, dt, ts(tk, 128)],
                                             rhs=wTs[2][:, dt, :],
                                             start=(dt == 0),
                                             stop=(dt == NK - 1))
                        vb = ar.tile([128, CH], BF, tag="vb")
                        nc.scalar.activation(vb[:], pv_[:], COPY)
                        nc.scalar.dma_start(
                            v_d[ds((pan * 2 + tk) * 128, 128), :], vb[:])

                    for j, (wi, ct) in enumerate(
                            ((0, 0), (1, 0), (0, 1), (1, 1),
                             (0, 2), (1, 2), (0, 3), (1, 3))):
                        qk_chain(wi, ct, qT_d if wi == 0 else kT_d)
                        if j >= 1:
                            swap_pend.pop(0)()
                    v_chain(0)
                    swap_pend.pop(0)()
                    v_chain(1)

            # ================= phase B: attention =================
            with tc.tile_pool(name="wof1", bufs=1) as wof1:
                # first wo half-quarter preloads during phase B on the
                # otherwise-idle sync queue (pool precedes at/att so its
                # space is not gated on their release)
                wqt0 = wof1.tile([128, NK, 512], BF, tag="wqt0")
                for cq in range(4):
                    nc.sync.dma_start(
                        wqt0[:, ds(cq * (NK // 4), NK // 4), :],
                        woTF.ap()[ds(cq * (D // 4), D // 4),
                                  ds(0, 512)].rearrange(
                            "(ct p) d -> p ct d", p=128))
                at_cm = tc.tile_pool(name="at", bufs=2)
                at = at_cm.__enter__()

                def _bh_loads(b_i, h):
                    qTb = at.tile([128, S], BF, tag="qTb", name="qTb")
                    nc.gpsimd.dma_start(
                        qTb[:], qT_d[ds(h * HD, HD), ds(b_i * S, S)])
                    kTb = at.tile([128, S], BF, tag="kTb", name="kTb")
                    nc.gpsimd.dma_start(
                        kTb[:], kT_d[ds(h * HD, HD), ds(b_i * S, S)])
                    vb2 = at.tile([128, S // 128, HD], BF, tag="vb2",
                                  name="vb2")
                    nc.gpsimd.dma_start(
                        vb2[:],
                        v_d[ds(b_i * S, S), ts(h, HD)].rearrange(
                            "(kt p) d -> p kt d", p=128))
                    return qTb, kTb, vb2

                with tc.tile_pool(name="att", bufs=2) as att, \
                     tc.tile_pool(name="ps_sg", bufs=3,
                                  space="PSUM") as ps_sg, \
                     tc.tile_pool(name="ps_o", bufs=1, space="PSUM") as ps_o, \
                     tc.tile_pool(name="ps_s2", bufs=1,
                                  space="PSUM") as ps_s2:

                    def emit_zip(fronts, backs):
                        # proportional interleave, front-first: score
                        # matmuls of chunk n cover chunk n-1's deps
                        nf, nb = len(fronts), len(backs)
                        fi = bi = 0
                        while fi < nf or bi < nb:
                            if fi < nf and (bi >= nb or fi * nb <= bi * nf):
                                fronts[fi]()
                                fi += 1
                            else:
                                backs[bi]()
                                bi += 1

                    pend = []      # chunk n-1's pv/adapter/tail closures
                    loads = None

                    for bh in range(B * H_LOC):
                        b_i, h = divmod(bh, H_LOC)
                        if bh == 0:
                            loads = _bh_loads(b_i, h)
                        qTb, kTb, vb2 = loads
                        if bh + 1 < B * H_LOC:
                            loads = _bh_loads(*divmod(bh + 1, H_LOC))

                        for qc in range(NQC):
                            nkt = (qc + 1) * 4
                            stb = att.tile([128, S // 128, 512], BF,
                                           tag="stb")
                            acc = att.tile([128, 2, 512], BF, tag="acc")
                            o_ps = ps_o.tile([128, 512], F32, tag="ops",
                                             name="o_ps")

                            def _score2(g, stb=stb, qTb=qTb, kTb=kTb, qc=qc):
                                sps = ps_sg.tile([128, 2, 512], F32,
                                                 tag="sps")
                                for j2 in range(2):
                                    nc.tensor.matmul(
                                        sps[:, j2, :],
                                        lhsT=kTb[:, ts(2 * g + j2, 128)],
                                        rhs=qTb[:, ts(qc, 512)],
                                        start=True, stop=True)
                                nc.scalar.activation(
                                    stb[:, 2 * g:2 * g + 2, :],
                                    sps[:], EXP, scale=SCALE)
                                if g // 2 == qc:
                                    j2 = (g % 2) * 2
                                    nc.vector.tensor_mul(
                                        stb[:, 2 * g:2 * g + 2, :],
                                        stb[:, 2 * g:2 * g + 2, :],
                                        m01_sb[:, ds(j2, 2), :])

                            state = {}

                            def _adp1(qTb=qTb, qc=qc, h=h, state=state):
                                s2 = ps_s2.tile([128, 512], F32, tag="s2")
                                state["s2"] = s2
                                nc.tensor.matmul(s2[64:64 + AL, :],
                                                 lhsT=akT_sb[:, h, :],
                                                 rhs=qTb[:, ts(qc, 512)],
                                                 start=True, stop=True,
                                                 tile_position=(0, 64))
                                pab = att.tile([AL, 512], BF, tag="pab",
                                               bufs=3)
                                nc.scalar.activation(pab[:],
                                                     s2[64:64 + AL, :],
                                                     EXP, scale=SCALE)
                                state["pab"] = pab

                            def _adp2(state=state):
                                nc.tensor.matmul(state["s2"][32:33, :],
                                                 lhsT=ones[:AL, 0:1],
                                                 rhs=state["pab"][:],
                                                 start=True, stop=True,
                                                 tile_position=(0, 32))

                            def _pv(kt, stb=stb, acc=acc, vb2=vb2,
                                    o_ps=o_ps, nkt=nkt):
                                # stop stays False: the scaled adapter PV
                                # joins this accumulation group in _oacc
                                nc.tensor.matmul(o_ps[:], lhsT=vb2[:, kt, :],
                                                 rhs=stb[:, kt, :],
                                                 start=(kt == 0),
                                                 stop=False)
                                if kt % 4 == 3:
                                    g4 = kt // 4
                                    pr = stb[:, 4 * g4:4 * g4 + 2,
                                             :].rearrange("p b a -> p (b a)")
                                    ps2 = stb[:, 4 * g4 + 2:4 * g4 + 4,
                                              :].rearrange("p b a -> p (b a)")
                                    wacc = acc[:].rearrange(
                                        "p b a -> p (b a)")
                                    if g4 == 0:
                                        nc.vector.tensor_add(wacc, pr, ps2)
                                    else:
                                        tmp = att.tile([128, 2, 512], BF,
                                                       tag="tmp")
                                        nc.vector.tensor_add(
                                            tmp[:].rearrange(
                                                "p b a -> p (b a)"),
                                            pr, ps2)
                                        nc.vector.tensor_add(
                                            wacc, wacc,
                                            tmp[:].rearrange(
                                                "p b a -> p (b a)"))

                            def _s2mm(acc=acc, state=state):
                                tr = att.tile([128, 512], BF, tag="tr")
                                nc.vector.tensor_add(tr[:], acc[:, 0, :],
                                                     acc[:, 1, :])
                                nc.tensor.matmul(state["s2"][0:1, :],
                                                 lhsT=ones[:, 0:1],
                                                 rhs=tr[:],
                                                 start=True, stop=True)

                            def _cmb(state=state):
                                # rows 0 (main sum d1) and 32 (adapter
                                # sum d2) of s2 -> reciprocals + copies
                                rs2f = att.tile([33, 512], F32, tag="rs2f")
                                nc.vector.reciprocal_approx_fast(
                                    rs2f[:], state["s2"][0:33, :])
                                rs2 = att.tile([33, 512], BF, tag="rs2")
                                nc.vector.tensor_copy(rs2[:], rs2f[:])
                                d1b = att.tile([1, 512], BF, tag="d1b")
                                nc.vector.tensor_copy(d1b[:],
                                                      state["s2"][0:1, :])
                                state["rs2"], state["d1b"] = rs2, d1b

                            def _t1(state=state):
                                # (1/d2) broadcast to AL partitions at
                                # base 0 (slot 0 of an sps tile) so the
                                # DVE multiply stays partition-aligned
                                bb = ps_sg.tile([128, 2, 512], F32,
                                                tag="sps", name="bcad1")
                                nc.tensor.matmul(bb[0:AL, 0, :],
                                                 lhsT=ones[32:33, 0:AL],
                                                 rhs=state["rs2"][32:33, :],
                                                 start=True, stop=True,
                                                 tile_position=(32, 0))
                                pabn = att.tile([AL, 512], BF, tag="pabn")
                                nc.vector.tensor_mul(pabn[:],
                                                     state["pab"][:],
                                                     bb[0:AL, 0, :])
                                state["pabn"], state["bb"] = pabn, bb

                            def _t2(state=state):
                                # d1 broadcast to AL partitions (slot 1)
                                bb = state["bb"]
                                nc.tensor.matmul(bb[0:AL, 1, :],
                                                 lhsT=ones[0:1, 0:AL],
                                                 rhs=state["d1b"][:],
                                                 start=True, stop=True,
                                                 tile_position=(0, 0))
                                pabn2 = att.tile([AL, 512], BF, tag="pabn2")
                                nc.vector.tensor_mul(pabn2[:],
                                                     state["pabn"][:],
                                                     bb[0:AL, 1, :])
                                state["pabn2"] = pabn2

                            def _oacc(h=h, o_ps=o_ps, state=state):
                                # scaled adapter PV closes the o_ps group:
                                # o_total = sum(p v) + av^T (pab d1/d2)
                                nc.tensor.matmul(o_ps[:],
                                                 lhsT=av_sb[:, ts(h, HD)],
                                                 rhs=state["pabn2"][:],
                                                 start=False, stop=True)

                            def _bc(state=state):
                                bb = ps_sg.tile([128, 2, 512], F32,
                                                tag="sps", name="bc")
                                nc.tensor.matmul(bb[:, 0, :],
                                                 lhsT=ones[0:1, :],
                                                 rhs=state["rs2"][0:1, :],
                                                 start=True, stop=True)
                                bcs = att.tile([128, 512], F32, tag="bcs")
                                nc.vector.tensor_copy(bcs[:], bb[:, 0, :])
                                state["bcs"] = bcs

                            def _fin(b_i=b_i, qc=qc, h=h, o_ps=o_ps,
                                     state=state):
                                ob = att.tile([128, 512], BF, tag="ob")
                                nc.vector.tensor_mul(ob[:], o_ps[:],
                                                     state["bcs"][:])
                                nc.scalar.dma_start(
                                    a2a_ins[b_i][2 * qc][ds(h * HD, HD), :],
                                    ob[:, 0:256])
                                nc.scalar.dma_start(
                                    a2a_ins[b_i][2 * qc + 1][ds(h * HD, HD),
                                                             :],
                                    ob[:, 256:512])

                            fronts = [lambda g=g, f=_score2: f(g)
                                      for g in range(nkt // 2)]
                            emit_zip(fronts, pend)

                            pvs = [lambda kt=kt, f=_pv: f(kt)
                                   for kt in range(nkt)]
                            pend = ([_adp1] + pvs[0:4] + [_adp2] + pvs[4:]
                                    + [_s2mm, _cmb, _t1, _t2, _oacc, _bc,
                                       _fin])

                        if h == H_LOC - 1:
                            # drain this batch, then exchange it
                            emit_zip([], pend)
                            pend = []
                            nc.gpsimd.collective_compute(
                                "AllToAll", mybir.AluOpType.bypass,
                                replica_groups=[list(range(N_CORES))],
                                ins=[a2a_ins[b_i].opt()],
                                outs=[a2a_outs[b_i].opt()])
                            for sc in range(N_CORES):
                                nc.gpsimd.dma_start(
                                    oTf[:, ds(sc * H_LOC, H_LOC),
                                        ds(b_i * 256, 256)],
                                    a2a_outs[b_i][sc].rearrange(
                                        "(c p) t -> p c t", p=128))

                at_cm.__exit__(None, None, None)
                # ========== phase C: wo in 512-column half-quarters ==========
                # dqh0 was preloaded during phase B (wof1); dqh1-3 stream
                # on sync, dqh4-7 on scalar (parallel queues).  yf copies
                # go to DVE and output spills to gpsimd so the scalar
                # queue stays clear for loads.
                with tc.tile_pool(name="wof", bufs=4) as wof, \
                     tc.tile_pool(name="wy", bufs=3) as wy, \
                     tc.tile_pool(name="ps_y", bufs=2, space="PSUM") as ps_y:
                    wqts = {0: wqt0}
                    for dqh in (1, 2, 3, 4, 5, 6, 7):
                        wqt = wof.tile([128, NK, 512], BF, tag="wqt",
                                       name=f"wqt{dqh}")
                        eng = nc.sync if dqh <= 3 else nc.scalar
                        for cq in range(4):
                            eng.dma_start(
                                wqt[:, ds(cq * (NK // 4), NK // 4), :],
                                woTF.ap()[ds(cq * (D // 4), D // 4),
                                          ds(dqh * 512, 512)].rearrange(
                                    "(ct p) d -> p ct d", p=128))
                        wqts[dqh] = wqt

                    def _chain(dqh, tt):
                        wqt = wqts[dqh]
                        yt = ps_y.tile([128, 512], F32, tag="yt")
                        for ct in range(NK):
                            nc.tensor.matmul(
                                yt[:],
                                lhsT=oTf[:, ct, ts(tt, 128)],
                                rhs=wqt[:, ct, :],
                                start=(ct == 0), stop=(ct == NK - 1))
                        yf = wy.tile([128, 512], F32, tag="yf")
                        nc.vector.tensor_copy(yf[:], yt[:])
                        nc.gpsimd.dma_start(
                            out.ap()[ds(tt * 128, 128), ds(dqh * 512, 512)],
                            yf[:])

                    # b0-token chains of dqh0-3 first so the b1 AllToAll
                    # hides under them; then their b1 tokens (frees the
                    # buffers for dqh5-7), then dqh4-7
                    for dqh, tt in ((0, 0), (0, 1), (1, 0), (1, 1),
                                    (2, 0), (2, 1), (3, 0), (3, 1),
                                    (0, 2), (0, 3), (1, 2), (1, 3),
                                    (2, 2), (2, 3), (3, 2), (3, 3),
                                    (4, 0), (4, 1), (4, 2), (4, 3),
                                    (5, 0), (5, 1), (5, 2), (5, 3),
                                    (6, 0), (6, 1), (6, 2), (6, 3),
                                    (7, 0), (7, 1), (7, 2), (7, 3)):
                        _chain(dqh, tt)
    nc.compile()
    return nc


_NC_CACHE = None


def _prep(x, wq, wk, wv, wo, gate, adapter, freqs_cos, freqs_sin, mask):
    """Host-side layout prep. Returns per-core input maps."""
    xf = np.asarray(x, np.float32).reshape(TOK, D)
    xT = np.ascontiguousarray(xf.T).astype(BF_NP)
    wq = np.asarray(wq, np.float32)
    wk = np.asarray(wk, np.float32)
    wv = np.asarray(wv, np.float32)
    wo = np.asarray(wo, np.float32)
    g = np.tanh(np.asarray(gate, np.float32).reshape(H))
    ad = np.asarray(adapter, np.float32).reshape(AL, D)
    a_k = ad @ wk.T          # [AL, H*HD]
    a_v = (ad @ wv.T) * np.repeat(g, HD)[None, :]   # gate folded in
    fc = np.asarray(freqs_cos, np.float32)          # [S, HD//2]
    fs = np.asarray(freqs_sin, np.float32)
    woTF = np.ascontiguousarray(wo.T).astype(BF_NP)
    mk = np.asarray(mask, np.float32).reshape(S, S)
    # RoPE tables in [ch, tok] orientation.  After the pair swap
    # (psw: c <-> c^1):  out[c] = q[c]*csT[c] + q[c^1]*snP[c^1].
    csT = np.empty((128, S), np.float32)
    snP = np.empty((128, S), np.float32)
    csT[0::2, :] = fc.T
    csT[1::2, :] = fc.T
    snP[0::2, :] = fs.T          # partner of odd lanes: +sin
    snP[1::2, :] = -fs.T         # partner of even lanes: -sin
    csT = np.ascontiguousarray(csT).astype(BF_NP)
    snP = np.ascontiguousarray(snP).astype(BF_NP)
    psw = np.zeros((128, 128), np.float32)
    idx = np.arange(128)
    psw[idx, idx ^ 1] = 1.0
    psw = psw.astype(BF_NP)
    # multiplicative 0/1 diagonal masks, S^T orientation: m01[j][k, q]
    m01 = np.empty((128, 4, 512), np.float32)
    for j in range(4):
        blk = mk[0:512, j * 128:(j + 1) * 128]    # [q, k] additive
        m01[:, j, :] = (blk == 0.0).T.astype(np.float32)
    m01 = np.ascontiguousarray(m01.reshape(128, 4 * 512)).astype(BF_NP)

    in_maps = []
    for r in range(N_CORES):
        sl = slice(r * CH, (r + 1) * CH)
        akr = a_k[:, sl]     # [AL, CH]
        akTl = np.zeros((HD, H_LOC, AL), np.float32)
        for h in range(H_LOC):
            akTl[:, h, :] = akr[:, h * HD:(h + 1) * HD].T
        in_maps.append({
            "xT": xT,
            "wqT": np.ascontiguousarray(wq[sl].T).astype(BF_NP),
            "wkT": np.ascontiguousarray(wk[sl].T).astype(BF_NP),
            "wvT": np.ascontiguousarray(wv[sl].T).astype(BF_NP),
            "woTF": woTF,
            "csT": csT,
            "snP": snP,
            "psw": psw,
            "akT": np.ascontiguousarray(
                akTl.reshape(HD, H_LOC * AL)).astype(BF_NP),
            "av": np.ascontiguousarray(a_v[:, sl]).astype(BF_NP),
            "m01": m01,
        })
    return in_maps


def kernel(x, wq, wk, wv, wo, gate, adapter, freqs_cos, freqs_sin, mask,
           start_pos=0, **_unused):
    global _NC_CACHE
    if _NC_CACHE is None:
        _NC_CACHE = build()
    nc = _NC_CACHE
    in_maps = _prep(x, wq, wk, wv, wo, gate, adapter,
                    freqs_cos, freqs_sin, mask)
    res = bass_utils.run_bass_kernel_spmd(nc, in_maps,
                                          core_ids=list(range(N_CORES)))
    # core r holds b0 tokens [r*256, r*256+256) then b1 tokens likewise
    y = np.empty((TOK, D), np.float32)
    for r in range(N_CORES):
        arr = np.asarray(res.results[r]["out"])
        y[r * 256:(r + 1) * 256] = arr[0:256]
        y[S + r * 256:S + (r + 1) * 256] = arr[256:512]
    return y.reshape(B, S, D)


if __name__ == "__main__":
    nc = build()
    print("compiled ok, instrs:",
          sum(len(bb.instructions) for f in nc.m.functions for bb in f.blocks))
